# revision 34
# baseline (speedup 1.0000x reference)
"""3-layer GAT (DGL GATConv semantics) on 8 Trainium2 NeuronCores.

v2 strategy (graph-parallel; see sim_v2.py for the numpy golden model):
  - Host load-balances dst nodes into 8*49 windows of 128 dsts (LPT packing);
    node order is permuted accordingly, output inverse-permuted at the end.
  - Per layer: each core projects its local nodes ([feat|el|er] in one bf16
    matmul), packs [feat bf16 | el f32] into a 768B G-row plus one dedicated
    pad row per core (feat=0, el=-60 so padded edges vanish in exp), and
    AllGathers G.
  - Edge phase per 128-dst window: dma_gather source G-rows (edges presorted
    by dst; 4 SWDGE queues round-robin so descriptor emission runs on all
    four Q7 core pairs), build ONE is_ge staircase mask B per tile, and use
    it twice with suffix-sum telescoping:
      * er per edge = B.T @ diff(er) (first-difference along dst rows,
        bf16 hi/lo split for accuracy),
      * per-dst segments = first-difference (bidiagonal matmul) of the
        suffix sums B.T-accumulated over tiles, with the softmax denominator
        fused as 4 extra matmul columns and the 1e-16 clamp as a rank-1
        epsilon matmul.
    LeakyReLU/exp/copies run on the Scalar engine; the Vector engine only
    does the mask build, logit add, weighted-feature multiply, and the
    normalization arithmetic. h stays resident in SBUF (bf16) across layers
    (it is both the next layer's input and its residual).
"""

from contextlib import ExitStack

import numpy as np

import concourse.bass as bass  # noqa: F401
import concourse.bacc as bacc
import concourse.mybir as mybir
import concourse.tile as tile
from concourse import bass_utils

F32 = mybir.dt.float32
BF16 = mybir.dt.bfloat16
I16 = mybir.dt.int16
F8 = mybir.dt.float8e4

GROW = 512  # f8 bytes per G row: [feat f8 256 | el-as-f32 16 | pad] = 512B
GUSE = 272  # used bytes per G row
GMAX = 1024


def _bf16(a):
    import ml_dtypes
    return np.asarray(a, np.float32).astype(ml_dtypes.bfloat16)


class Cfg:
    def __init__(self, n, e, fin, h, dh, ncores, wpc, nchunk=2):
        self.N, self.E, self.FIN, self.H, self.DH = n, e, fin, h, dh
        self.HID = h * dh
        self.NCORES, self.WPC = ncores, wpc
        self.NPC = wpc * 128            # nodes per core (excl. pad rows)
        self.NPAD = ncores * self.NPC   # padded global nodes
        assert self.NPAD >= n
        self.KT = self.HID // 128       # K tiles for layers 1,2
        self.KT0 = fin // 128           # K tiles for layer 0
        # AllGather chunking: the per-layer G table is split into NCHUNK
        # independent Shared tables (one collective each) so chunk k's
        # AllGather is issued as soon as its windows are projected and
        # overlaps the remaining proj/agg work. Each chunk table is
        # < 32768 rows so int16 gather indices address it directly; each
        # chunk carries one pad row per core (el=-60) for gather tails.
        self.NCHUNK = nchunk
        base = wpc // nchunk
        wsplit = [base + (1 if i < wpc % nchunk else 0)
                  for i in range(nchunk)]
        self.CH_WEND = np.cumsum(wsplit).tolist()   # window end per chunk
        self.CH_RST = [0] + [we * 128 for we in self.CH_WEND[:-1]]
        self.CH_NR = [we * 128 - rs                 # node rows per chunk
                      for we, rs in zip(self.CH_WEND, self.CH_RST)]
        self.CH_RSZ1 = [nr + 1 for nr in self.CH_NR]  # + per-core pad row
        # g_loc layout: chunk k's node rows then its pad row
        self.GL_OFF = np.concatenate(
            [[0], np.cumsum(self.CH_RSZ1)]).astype(np.int64).tolist()
        self.GL_ROWS = self.GL_OFF[-1]              # NPC + NCHUNK
        self.G_SZ = [ncores * sz for sz in self.CH_RSZ1]  # chunk table rows
        assert all(sz <= 32768 for sz in self.G_SZ)
        self.PADK = self.CH_NR                      # core-0 pad idx per chunk


def _lpt_windows(deg, cfg):
    """Assign nodes to ncores*wpc windows of exactly 128 slots, balancing
    per-window edge counts (LPT greedy). Returns perm_pos[node] -> slot."""
    import heapq

    nw = cfg.NCORES * cfg.WPC
    order = np.argsort(-deg, kind="stable")
    heap = [(0, w) for w in range(nw)]
    heapq.heapify(heap)
    counts = np.zeros(nw, np.int64)
    wsum = np.zeros(nw, np.int64)
    assign = np.empty(cfg.N, np.int64)
    for n in order:
        while True:
            s, w = heapq.heappop(heap)
            if counts[w] < 128:
                break
        assign[n] = w
        counts[w] += 1
        wsum[w] += deg[n]
        if counts[w] < 128:
            heapq.heappush(heap, (wsum[w], w))
    perm_pos = np.empty(cfg.N, np.int64)
    next_row = np.zeros(nw, np.int64)
    for n in range(cfg.N):
        w = assign[n]
        perm_pos[n] = w * 128 + next_row[w]
        next_row[w] += 1
    return perm_pos


def _wrap16(vals, cap):
    """Wrap a list of idx values into the Q7 [16, cap//16] layout."""
    out = np.zeros((16, cap // 16), np.int16)
    j = np.arange(len(vals))
    out[j % 16, j // 16] = vals.astype(np.int16)
    return out


def preprocess(x, edge_index, cfg):
    src = np.asarray(edge_index[0], dtype=np.int64)
    dst = np.asarray(edge_index[1], dtype=np.int64)
    deg = np.bincount(dst, minlength=cfg.N)
    perm_pos = _lpt_windows(deg, cfg)

    psrc = perm_pos[src]
    pdst = perm_pos[dst]
    ncores, wpc, K = cfg.NCORES, cfg.WPC, cfg.NCHUNK
    s_core = psrc // cfg.NPC
    s_row = psrc % cfg.NPC
    reg = np.searchsorted(np.asarray(cfg.CH_RST), s_row, side="right") - 1
    rsz1 = np.asarray(cfg.CH_RSZ1)
    rst = np.asarray(cfg.CH_RST)
    g_of = s_core * rsz1[reg] + (s_row - rst[reg])  # idx within chunk table
    gw = pdst // 128
    row = pdst % 128

    core = gw // wpc
    wi = gw % wpc

    ek = (core * wpc + wi) * K + reg
    sort_idx = np.lexsort((g_of, ek))
    s_g = g_of[sort_idx]
    s_r = row[sort_idx]
    s_ek = ek[sort_idx]
    grp_start = np.searchsorted(s_ek, np.arange(ncores * wpc * K + 1))

    reg_t = np.zeros((K, wpc), np.int64)
    for w in range(wpc):
        for c in range(ncores):
            for k in range(K):
                g = (c * wpc + w) * K + k
                ne = grp_start[g + 1] - grp_start[g]
                reg_t[k, w] = max(reg_t[k, w], (ne + 127) // 128)
    T = reg_t.sum(axis=0)
    TT = int(T.sum())
    toff = np.concatenate([[0], np.cumsum(T)]).astype(np.int64)
    nidx = TT * 8

    idx16 = np.zeros((ncores, 16, nidx), np.int16)
    # st defaults to 128 so tiles a window reserves beyond this core's own
    # group tile count get an all-zero mask (B = e >= 128 never true).
    st = np.full((ncores, 128, TT), 128.0, np.float32)
    # dstrow defaults to -1 so unwritten tiles get an all-zero edge mask
    # (d <= dstrow never true); pad edge slots also use -1.
    dstrow = np.full((ncores, 128, TT), -1.0, np.float32)

    for c in range(ncores):
        for w in range(wpc):
            rt0 = toff[w]
            for k in range(K):
                g = (c * wpc + w) * K + k
                lo, hi_ = grp_start[g], grp_start[g + 1]
                e_g = s_g[lo:hi_]
                e_r = s_r[lo:hi_]
                nt = reg_t[k, w]
                if nt == 0:
                    assert len(e_g) == 0
                    continue
                cap = nt * 128
                n_e = len(e_g)
                vals = np.full(cap, cfg.PADK[k], np.int64)
                vals[:n_e] = e_g
                assert (vals >= 0).all() and (vals < cfg.G_SZ[k]).all()
                idx16[c, :, rt0 * 8:(rt0 + nt) * 8] = _wrap16(vals, cap)
                rows = np.full(cap, -1, np.int64)
                rows[:n_e] = e_r
                dstrow[c, :, rt0:rt0 + nt] = rows.reshape(nt, 128).T
                seg = np.searchsorted(e_r, np.arange(129))
                for tl in range(nt):
                    b = tl * 128
                    st[c, :, rt0 + tl] = np.clip(seg[:-1] - b, 0, 128)
                rt0 += nt

    xT = np.zeros((ncores, cfg.FIN, cfg.NPC), np.float32)
    inv_rows = np.full(cfg.NPAD, -1, np.int64)
    inv_rows[perm_pos] = np.arange(cfg.N)
    xf = np.asarray(x, np.float32)
    for c in range(ncores):
        sl = inv_rows[c * cfg.NPC:(c + 1) * cfg.NPC]
        valid = sl >= 0
        xc = np.zeros((cfg.NPC, cfg.FIN), np.float32)
        xc[valid] = xf[sl[valid]]
        xT[c] = xc.T

    # HW Q7 cores each read their own 16-partition group: replicate.
    idx16 = np.tile(idx16, (1, 8, 1))

    # host-precomputed masks (layer-invariant graph structure):
    #  bm[d, t, e]  = (e >= st[d, t])      dst-partitioned staircase
    #  bme[e, t, d] = (d <= dstrow[e, t])  edge-partitioned dual
    import ml_dtypes
    ar = np.arange(128, dtype=np.float32)
    bm = (dstrow.transpose(0, 2, 1)[:, None, :, :] >= ar[None, :, None,
                                                          None]
          ).astype(ml_dtypes.float8_e4m3fn)
    bme = (dstrow[:, :, :, None] >= ar[None, None, None, :]
           ).astype(ml_dtypes.float8_e4m3fn)

    return dict(perm_pos=perm_pos, reg_t=reg_t, T=T, TT=TT,
                toff=toff, nidx=nidx, idx16=idx16, bm=bm, bme=bme,
                dstrow=dstrow, xT=xT)


def pack_weights(cfg, Ws, als, ars, bs, resW0):
    # d-major feature layout on device: feature column (h*DH + d) lives at
    # position (d*H + h), so the per-head attention broadcasts are inner
    # stride-1 on the DVE (2x mode).
    H, DH, HID = cfg.H, cfg.DH, cfg.HID
    perm = np.empty(HID, np.int64)  # perm[new] = old
    for d in range(DH):
        for h in range(H):
            perm[d * H + h] = h * DH + d
    kts = []
    for l, W in enumerate(Ws):
        ALf = np.zeros((HID, H), np.float32)
        ARf = np.zeros((HID, H), np.float32)
        for h in range(H):
            ALf[h * DH:(h + 1) * DH, h] = als[l][h]
            ARf[h * DH:(h + 1) * DH, h] = ars[l][h]
        W = np.asarray(W, np.float32)
        if l > 0:
            # input h arrives d-major: permute contraction rows only
            # (W's columns — the feat axis ALf/ARf contract — stay original)
            W = W[perm, :]
        Wc = np.concatenate([W[:, perm], W @ ALf, W @ ARf], axis=1)
        fin = W.shape[0]
        for k in range(fin // 128):
            kts.append(Wc[k * 128:(k + 1) * 128])
    w_all = np.transpose(np.stack(kts), (1, 0, 2)).copy()  # [128, nk, 264]
    b_rep = np.stack([np.tile(b[None, perm], (128, 1)) for b in bs], axis=1)
    has_b = any(np.abs(b).max() > 0 for b in bs)
    rw = np.asarray(resW0, np.float32)[:, perm]
    return w_all, b_rep, rw, has_b


def _consts(cfg):
    # D: pm2[d] = pm[d] - pm[d+1]  (lhsT[k, d]: +1 at k=d, -1 at k=d+1)
    D = np.zeros((128, 128), np.float32)
    Dr = np.zeros((128, 128), np.float32)
    for d in range(128):
        D[d, d] = 1.0
        if d + 1 < 128:
            D[d + 1, d] = -1.0
        # Dr: er_diff[d] = er[d] - er[d-1]
        Dr[d, d] = 1.0
        if d - 1 >= 0:
            Dr[d - 1, d] = -1.0
    ones = np.ones((1, 128), np.float32)
    eps = np.zeros((1, 264), np.float32)
    eps[0, 256:260] = 1e-16
    # pad G-row: feat 0, el = -60.0 (4x f32) at bytes 256:272
    pad = np.zeros(GROW, np.uint8)
    el = np.full(4, -60.0, np.float32)
    pad[256:272] = el.view(np.uint8)
    import ml_dtypes
    padrow = pad.view(ml_dtypes.float8_e4m3fn).reshape(1, GROW)
    return D, Dr, ones, eps, padrow


def _gather(nc, out_ap, in_ap, idx_sb, col0, n, elem, q=0, sem=None):
    """One dma_gather prep for n indices. q selects the SWDGE queue: queue
    q's descriptors are emitted by Q7 core pair (2q, 2q+1), so spreading
    windows across queues parallelizes descriptor generation 4x. With
    prepare_only the Q7 pair is released once descriptors are written (it
    does not hold through the SDMA drain); the caller fires trigger_dma."""
    nc.gpsimd.dma_gather(
        out_ap=out_ap, in_ap=in_ap,
        idxs_ap=idx_sb[:, col0: col0 + n // 16],
        num_idxs=n, num_idxs_reg=n, elem_size=elem,
        single_packet=(n <= GMAX), queue_num=q)


def build_program(cfg, meta, num_cores, has_b=False, dbg_layer=None):
    nc = bacc.Bacc("TRN2", target_bir_lowering=False, debug=False,
                   num_devices=num_cores, num_swdge_queues=4)
    NPC, HID = cfg.NPC, cfg.HID
    WPC = cfg.WPC
    NK = cfg.KT0 + 2 * cfg.KT
    reg_t, T, toff, TT = meta["reg_t"], meta["T"], meta["toff"], meta["TT"]
    maxT = int(T.max())
    AF = mybir.ActivationFunctionType

    d_xT = nc.dram_tensor("xT", [cfg.FIN, NPC], BF16, kind="ExternalInput")
    d_w = nc.dram_tensor("w_all", [128, NK, 264], BF16, kind="ExternalInput")
    d_rw = nc.dram_tensor("resW0", [128, 256], BF16, kind="ExternalInput")
    d_id = nc.dram_tensor("ident", [128, 128], BF16, kind="ExternalInput")
    d_D = nc.dram_tensor("Dmat", [128, 128], F32, kind="ExternalInput")
    d_Dr = nc.dram_tensor("Drmat", [128, 128], F32, kind="ExternalInput")
    d_ones = nc.dram_tensor("ones1", [1, 128], F32, kind="ExternalInput")
    d_eps = nc.dram_tensor("epsrow", [1, 264], F32, kind="ExternalInput")
    d_pad = nc.dram_tensor("padrow", [1, GROW], F8, kind="ExternalInput")
    d_idx = nc.dram_tensor("idx16", [128, meta["nidx"]], I16,
                           kind="ExternalInput")
    d_bm = nc.dram_tensor("bm", [128, TT, 128], F8, kind="ExternalInput")
    d_bme = nc.dram_tensor("bme", [128, TT, 128], F8, kind="ExternalInput")
    d_out = nc.dram_tensor("out", [NPC, cfg.DH], F32, kind="ExternalOutput")
    if has_b:
        d_b = nc.dram_tensor("b_rep", [128, 3, 256], F32,
                             kind="ExternalInput")
    if dbg_layer is not None:
        d_dbg = nc.dram_tensor("dbg", [NPC, HID], F32, kind="ExternalOutput")
        d_dbg2 = nc.dram_tensor("dbg2", [NPC, 524], F32,
                                kind="ExternalOutput")
        d_dbg3 = nc.dram_tensor("dbg3", [128, maxT * 128], F32,
                                kind="ExternalOutput")
        d_dbg4 = nc.dram_tensor("dbg4", [128, maxT * 8], F32,
                                kind="ExternalOutput")

    with ExitStack() as ctx:
        tc = ctx.enter_context(tile.TileContext(nc))
        cpool = ctx.enter_context(tc.tile_pool(name="const", bufs=1))
        dram = ctx.enter_context(tc.tile_pool(name="dram", bufs=1,
                                              space="DRAM"))
        fgpool = ctx.enter_context(tc.tile_pool(name="fg", bufs=6))
        bpool = ctx.enter_context(tc.tile_pool(name="bm", bufs=6))
        epool = ctx.enter_context(tc.tile_pool(name="e", bufs=3))
        wpool = ctx.enter_context(tc.tile_pool(name="wt", bufs=3))
        hpool = ctx.enter_context(tc.tile_pool(name="h", bufs=4))
        gpool = ctx.enter_context(tc.tile_pool(name="g", bufs=3))
        spool = ctx.enter_context(tc.tile_pool(name="s", bufs=3))
        ps_m = ctx.enter_context(tc.tile_pool(name="psm", bufs=2,
                                              space="PSUM"))
        ps_m2 = ctx.enter_context(tc.tile_pool(name="psm2", bufs=1,
                                               space="PSUM"))
        # proj matmul outputs double-buffered: layer 0's per-window chain
        # (matmul -> copies) pipelines across windows instead of
        # serializing on a single PSUM bank.
        ps_p = ctx.enter_context(tc.tile_pool(name="psp", bufs=2,
                                              space="PSUM"))
        ps_t = ctx.enter_context(tc.tile_pool(name="pst", bufs=1,
                                              space="PSUM"))
        ps_e = ctx.enter_context(tc.tile_pool(name="pse", bufs=1,
                                              space="PSUM"))
        ps_q = ctx.enter_context(tc.tile_pool(name="psq", bufs=1,
                                              space="PSUM"))

        g_loc = dram.tile([cfg.GL_ROWS, GROW], F8)
        g_fulls = [
            [dram.tile([cfg.G_SZ[k], GROW], F8, name=f"g_full{i}_{k}",
                       addr_space="Shared" if num_cores > 4 else "Local")
             for k in range(cfg.NCHUNK)]
            for i in range(3)]

        # ---- persistent SBUF state ----
        xT_sb = cpool.tile([128, NPC], BF16)
        nc.sync.dma_start(xT_sb[:], d_xT[:])
        w_sb = cpool.tile([128, NK, 264], BF16)
        nc.sync.dma_start(w_sb[:], d_w[:])
        rw_sb = cpool.tile([128, 256], BF16)
        nc.sync.dma_start(rw_sb[:], d_rw[:])
        id_sb = cpool.tile([128, 128], BF16)
        nc.sync.dma_start(id_sb[:], d_id[:])
        D_sb = cpool.tile([128, 128], F32)
        nc.sync.dma_start(D_sb[:], d_D[:])
        Dr_sb = cpool.tile([128, 128], F32)
        nc.sync.dma_start(Dr_sb[:], d_Dr[:])
        ones_sb = cpool.tile([1, 128], F32)
        nc.sync.dma_start(ones_sb[:], d_ones[:])
        eps_sb = cpool.tile([1, 264], F32)
        nc.sync.dma_start(eps_sb[:], d_eps[:])
        pad_sb = cpool.tile([1, GROW], F8)
        nc.sync.dma_start(pad_sb[:], d_pad[:])
        idx_sb = cpool.tile([128, meta["nidx"]], I16)
        nc.sync.dma_start(idx_sb[:], d_idx[:])
        if has_b:
            b_sb = cpool.tile([128, 3, 256], F32)
            nc.sync.dma_start(b_sb[:], d_b[:])
        er_res = cpool.tile([128, WPC, 4], F32)
        h_keep = cpool.tile([128, WPC, HID], BF16)
        qctr = [0]
        # per-chunk pad rows are layer-invariant: write once
        for k in range(cfg.NCHUNK):
            nc.sync.dma_start(
                g_loc[cfg.GL_OFF[k + 1] - 1:cfg.GL_OFF[k + 1], :],
                pad_sb[:])
        chunk_of_w = np.searchsorted(np.asarray(cfg.CH_WEND),
                                     np.arange(WPC), side="right")

        kt_of_layer = [list(range(cfg.KT0)),
                       list(range(cfg.KT0, cfg.KT0 + cfg.KT)),
                       list(range(cfg.KT0 + cfg.KT, NK))]

        def proj_window(l, nt):
            kts = kt_of_layer[l]
            lhsTs = []
            if l == 0:
                lhsTs.append(xT_sb[:, nt * 128:(nt + 1) * 128])
            else:
                for ft in range(cfg.KT):
                    pst = ps_t.tile([128, 128], BF16, tag="pt")
                    nc.tensor.transpose(
                        pst[:],
                        h_keep[:, nt, ft * 128:(ft + 1) * 128],
                        id_sb[:])
                    hT = hpool.tile([128, 128], BF16, tag="lhsT")
                    nc.scalar.copy(hT[:], pst[:])
                    lhsTs.append(hT[:])
            pp = ps_p.tile([128, 264], F32, tag="pp")
            for k, (kt, lt) in enumerate(zip(kts, lhsTs)):
                nc.tensor.matmul(pp[:], lt, w_sb[:, kt, :],
                                 start=(k == 0), stop=(k == len(kts) - 1))
            g_sb = gpool.tile([128, GUSE], F8)
            nc.scalar.copy(g_sb[:, 0:256], pp[:, 0:256])
            nc.scalar.copy(g_sb[:, 256:272].bitcast(F32), pp[:, 256:260])
            nc.scalar.copy(er_res[:, nt, :], pp[:, 260:264])
            go = nt * 128 + int(chunk_of_w[nt])
            nc.sync.dma_start(g_loc[go:go + 128, 0:GUSE],
                              g_sb[:, 0:GUSE])
            if l == 0:
                pr = ps_p.tile([128, 256], F32, tag="pp")
                nc.tensor.matmul(pr[:], lhsTs[0], rw_sb[:],
                                 start=True, stop=True)
                # layer-0 residual (x @ resW0) parked in h_keep; DVE copy
                # keeps the exposed layer-0 proj chain off the Scalar engine
                nc.vector.tensor_copy(h_keep[:, nt, :], pr[:])


        def issue_chunk(l, k):
            """AllGather chunk k of layer l's G table; issued as soon as
            its windows are projected so the collective overlaps remaining
            proj/agg work."""
            nc.gpsimd.collective_compute(
                "AllGather", mybir.AluOpType.bypass,
                replica_groups=[list(range(num_cores))],
                ins=[g_loc[cfg.GL_OFF[k]:cfg.GL_OFF[k + 1], :].opt()],
                outs=[g_fulls[l][k].opt()])

        def agg_window(l, w):
            Tw = int(T[w])
            t0 = int(toff[w])

            # er first-difference with bf16 hi/lo split
            er_dq = ps_q.tile([128, 4], F32, tag="edq")
            nc.tensor.matmul(er_dq[:], Dr_sb[:], er_res[:, w, :],
                             start=True, stop=True)
            er8 = epool.tile([128, 8], BF16, tag="er8")
            nc.scalar.copy(er8[:, 0:4], er_dq[:])
            nc.vector.tensor_tensor(er8[:, 4:8], er_dq[:], er8[:, 0:4],
                                    mybir.AluOpType.subtract)

            fg = fgpool.tile([128, maxT, GROW], F8)
            off = 0
            for k in range(cfg.NCHUNK):
                ntk = int(reg_t[k, w])
                if ntk > 0:
                    _gather(nc, fg[:, off:off + ntk, :],
                            g_fulls[l][k][:, :], idx_sb,
                            (t0 + off) * 8, ntk * 128, GROW,
                            q=qctr[0] % 4)
                    qctr[0] += 1
                    off += ntk

            # host-precomputed masks (layer-invariant):
            #  bm[d, t, e] = (e >= st[d, t])   for the er matmul
            #  bme[e, t, d] = (d <= dstrow)    for the aggregation matmul
            bm = bpool.tile([128, maxT, 128], F8, tag="bmd")
            nc.sync.dma_start(bm[:, 0:Tw, :], d_bm[:, t0:t0 + Tw, :])
            bme = bpool.tile([128, maxT, 128], F8, tag="bme")
            nc.sync.dma_start(bme[:, 0:Tw, :], d_bme[:, t0:t0 + Tw, :])

            # er per edge via telescoping (hi+lo columns)
            pe8 = ps_e.tile([128, maxT, 8], F32, tag="pe8")
            for t in range(Tw):
                nc.tensor.matmul(pe8[:, t, :], bm[:, t, :], er8[:],
                                 start=True, stop=True)

            # logits + LeakyReLU + exp (exp lands in wsb cols 256:260)
            el_v = fg[:, 0:Tw, 256:272].bitcast(F32)    # [128, Tw, 4]
            e_sb = epool.tile([128, maxT, 4], F32, tag="e")
            nc.vector.tensor_tensor(e_sb[:, 0:Tw, :], el_v,
                                    pe8[:, 0:Tw, 0:4],
                                    mybir.AluOpType.add)
            nc.vector.tensor_tensor(e_sb[:, 0:Tw, :], e_sb[:, 0:Tw, :],
                                    pe8[:, 0:Tw, 4:8],
                                    mybir.AluOpType.add)
            nc.vector.scalar_tensor_tensor(
                e_sb[:, 0:Tw, :], e_sb[:, 0:Tw, :], 0.2, e_sb[:, 0:Tw, :],
                mybir.AluOpType.mult, mybir.AluOpType.max)
            # exp into a compact tile (the DVE multiply needs a dense
            # operand to stay in 2x mode); the Scalar engine also copies it
            # into wsb's denominator columns -- a DVE copy would run in
            # 4x/2-port mode and lock GpSimd out of the shared SBUF port,
            # starving gather descriptor generation.
            exb = epool.tile([128, maxT, 4], BF16, tag="exb")
            nc.scalar.activation(exb[:, 0:Tw, :], e_sb[:, 0:Tw, :],
                                 AF.Exp)
            wsb = wpool.tile([128, maxT, 260], BF16)
            nc.scalar.copy(wsb[:, 0:Tw, 256:260], exb[:, 0:Tw, :])

            # weighted feats (d-major): wsb[e,t,d,h] = fg[e,t,d,h]*ex[e,t,h]
            # inner stride-1 on h for both sources -> DVE 2x mode
            nc.vector.tensor_tensor(
                wsb[:, 0:Tw, 0:256].rearrange("p t (d h) -> p t d h",
                                              h=4),
                fg[:, 0:Tw, 0:256].rearrange("p t (d h) -> p t d h", h=4),
                exb[:, 0:Tw, :].unsqueeze(2).broadcast_to(
                    [128, Tw, 64, 4]),
                mybir.AluOpType.mult)

            # suffix sums over tiles: pm[d] = sum_t BT_t.T @ wsb_t
            pm = ps_m.tile([128, 260], F32)
            for t in range(Tw):
                nc.tensor.matmul(pm[:], bme[:, t, :], wsb[:, t, :],
                                 start=(t == 0), stop=(t == Tw - 1),
                                 skip_group_check=True)

            # telescope to per-dst segments + epsilon clamp
            s_sb = spool.tile([128, 260], F32)
            nc.scalar.copy(s_sb[:], pm[:])
            pm2 = ps_m2.tile([128, 260], F32)
            nc.tensor.matmul(pm2[:], D_sb[:], s_sb[:],
                             start=True, stop=False)
            nc.tensor.matmul(pm2[:], ones_sb[:], eps_sb[0:1, 0:260],
                             start=False, stop=True)

            rden = epool.tile([128, 4], F32, tag="rden")
            nc.vector.reciprocal(rden[:], pm2[:, 256:260])
            hn = hpool.tile([128, HID], F32, tag="hn")
            nc.vector.tensor_tensor(
                hn[:].rearrange("p (d h) -> p d h", h=4),
                pm2[:, 0:256].rearrange("p (d h) -> p d h", h=4),
                rden[:].unsqueeze(1).broadcast_to([128, 64, 4]),
                mybir.AluOpType.mult)
            nc.vector.tensor_tensor(hn[:], hn[:], h_keep[:, w, :],
                                    mybir.AluOpType.add)
            if has_b:
                nc.vector.tensor_tensor(hn[:], hn[:], b_sb[:, l, :],
                                        mybir.AluOpType.add)
            if dbg_layer is not None and l == dbg_layer and w == 0:
                d3 = hpool.tile([128, maxT * 128], F32, tag="d3sb")
                nc.vector.tensor_copy(
                    d3[:, 0:maxT * 4].rearrange("p (t e) -> p t e",
                                                t=maxT),
                    fg[:, 0:maxT, 256:272].bitcast(F32))
                nc.vector.tensor_copy(
                    d3[:, maxT * 4:maxT * 4 + maxT * 64]
                    .rearrange("p (t e) -> p t e", t=maxT),
                    fg[:, 0:maxT, 0:64])
                nc.sync.dma_start(d_dbg3[:], d3[:])
                d4 = hpool.tile([128, maxT * 8], F32, tag="d4sb")
                nc.vector.tensor_copy(
                    d4[:].rearrange("p (t e) -> p t e", t=maxT),
                    pe8[:, 0:maxT, :])
                nc.sync.dma_start(d_dbg4[:], d4[:])
            if dbg_layer is not None and l == dbg_layer:
                dsb = hpool.tile([128, HID], F32, tag="dsb")
                nc.vector.tensor_copy(dsb[:], hn[:])
                nc.sync.dma_start(d_dbg[w * 128:(w + 1) * 128, :], dsb[:])
                d2 = hpool.tile([128, 524], F32, tag="d2sb")
                nc.vector.tensor_copy(d2[:, 0:260], pm2[:, 0:260])
                nc.vector.tensor_copy(d2[:, 260:264], er_res[:, w, :])
                nc.vector.tensor_copy(d2[:, 264:524], s_sb[:])
                nc.sync.dma_start(d_dbg2[w * 128:(w + 1) * 128, :],
                                  d2[:])
            if l < 2:
                nc.scalar.activation(h_keep[:, w, :], hn[:], AF.Relu)
            else:
                osb = hpool.tile([128, cfg.DH], F32, tag="osb")
                nc.vector.tensor_reduce(
                    osb[:],
                    hn[:].rearrange("p (d h) -> p d h", h=4),
                    mybir.AxisListType.X, mybir.AluOpType.add)
                nc.scalar.mul(osb[:], osb[:], 1.0 / cfg.H)
                nc.sync.dma_start(d_out[w * 128:(w + 1) * 128, :], osb[:])


        chunk_of_wend = {we - 1: k for k, we in enumerate(cfg.CH_WEND)}
        for nt in range(WPC):
            proj_window(0, nt)
            if nt in chunk_of_wend:
                issue_chunk(0, chunk_of_wend[nt])
        for l in range(3):
            for w in range(WPC):
                agg_window(l, w)
                if l < 2:
                    proj_window(l + 1, w)
                    if w in chunk_of_wend:
                        issue_chunk(l + 1, chunk_of_wend[w])

    nc.compile()
    return nc


def make_in_maps(cfg, meta, wnp, num_cores):
    w_all, b_rep, rw, has_b = wnp
    D, Dr, ones, eps, padrow = _consts(cfg)
    ident = np.eye(128, dtype=np.float32)
    maps = []
    for c in range(num_cores):
        m = {
            "xT": _bf16(meta["xT"][c]),
            "w_all": _bf16(w_all), "resW0": _bf16(rw), "ident": _bf16(ident),
            "Dmat": D, "Drmat": Dr, "ones1": ones, "epsrow": eps,
            "padrow": padrow,
            "idx16": meta["idx16"][c],
            "bm": meta["bm"][c],
            "bme": meta["bme"][c],
        }
        if has_b:
            m["b_rep"] = b_rep.astype(np.float32)
        maps.append(m)
    return maps


def assemble_output(cfg, meta, results):
    out = np.empty((cfg.N, cfg.DH), np.float32)
    full = np.concatenate([r["out"] for r in results], axis=0)
    out[:] = full[meta["perm_pos"]]
    return out


def solve(x, edge_index, Ws, als, ars, bs, resW0, cfg, trace=False):
    meta = preprocess(x, edge_index, cfg)
    wnp = pack_weights(cfg, Ws, als, ars, bs, resW0)
    nc = build_program(cfg, meta, cfg.NCORES, has_b=wnp[3])
    in_maps = make_in_maps(cfg, meta, wnp, cfg.NCORES)
    res = bass_utils.run_bass_kernel_spmd(
        nc, in_maps, core_ids=list(range(cfg.NCORES)), trace=trace)
    out = assemble_output(cfg, meta, res.results)
    return out, res


def kernel(x, edge_index, W0, W1, W2, al0, al1, al2, ar0, ar1, ar2,
           b0, b1, b2, resW0):
    cfg = Cfg(n=50000, e=800000, fin=128, h=4, dh=64, ncores=8, wpc=49)
    out, _ = solve(np.asarray(x, np.float32), np.asarray(edge_index),
                   [np.asarray(W0, np.float32), np.asarray(W1, np.float32),
                    np.asarray(W2, np.float32)],
                   [np.asarray(al0, np.float32), np.asarray(al1, np.float32),
                    np.asarray(al2, np.float32)],
                   [np.asarray(ar0, np.float32), np.asarray(ar1, np.float32),
                    np.asarray(ar2, np.float32)],
                   [np.asarray(b0, np.float32), np.asarray(b1, np.float32),
                    np.asarray(b2, np.float32)],
                   np.asarray(resW0, np.float32), cfg)
    return out

